# revision 2
# baseline (speedup 1.0000x reference)
"""Trainium2 Bass kernel for dilated sliding-window attention (AttnWrapper).

Reference computation (all fp32):
  combined = [begin | main | end]                       # [8256, 768]
  keys[t]  = combined[t + 32 + off], off in +-{4..32}   # 16 dilated window keys
  q = (main @ wq.T + bq) * 96**-0.5
  k/v = keys @ w{k,v}.T + b{k,v}
  attn = softmax(q.k), ctx = attn.v, out = [main | ctx @ wo.T + bo]

Sharding: tokens across 8 cores (1024 each) with a 64-row halo of the
combined buffer; weights replicated. Each core computes attn_outT
[768, 1024]; the host transposes and concatenates with main.

Device-side notes (v2 — all-bf16 matmul pipeline):
 - All matmul operands bf16 (x, wq, wk, wv, wo cast on host). bf16 runs
   1 cycle/col at any size on PE (f32r needs >=256 cols), so the score
   matmuls can be restricted to each key-chunk's valid token window
   (128/192/192/192/64 of 512) instead of streaming all 512 columns.
 - bk dropped (softmax shift invariance); bv folded into bo2 = wo@bv + bo.
 - q scale and bq folded into wq/bq on the host.
 - A ones-column appended to each V head block makes the ctx matmul also
   produce the softmax denominator (row 96 of the ctx PSUM tile).
 - Normalization is per-(group,head) and fully pipelined: 1/denom =
   exp(-ln d) on ACT, partition_broadcast on GPSIMD, multiply on DVE,
   each overlapping the next head's PE work.
 - DMAs are batched (one per tensor, 3D access patterns) and ordered so
   the first projection matmul can start ~10us after kernel entry.
"""

import numpy as np

EMBED_DIM = 768
NUM_HEADS = 8
HEAD_DIM = 96
OVERLAP = 32
HALO = 2 * OVERLAP          # 64 extra combined rows per core
N_LINES = 8192
N_CORES = 8
TOK = N_LINES // N_CORES    # 1024 tokens per core
ROWS = TOK + HALO           # 1088 combined rows per core
GRP = 512                   # tokens per attention group
NG = TOK // GRP             # 2 groups
# key chunks (start, end), valid token windows (w0, w1), mask index
CHUNKS = [(0, 128, 0, 128, 0), (128, 256, 64, 256, 1), (256, 384, 192, 384, 1),
          (384, 512, 320, 512, 1), (512, 576, 448, 512, 2)]
VBLK = HEAD_DIM + 1         # 97: v head block + ones column
KC = EMBED_DIM // 128       # 6 contraction chunks of 128
NVC = (ROWS + 127) // 128   # 9 v row-chunks (8x128 + 64)
NCONST = 8 + 6 + 192        # bq | bo2 | masks(bf16 pairs)


def _build_program():
    import concourse.bacc as bacc
    import concourse.mybir as mybir
    from concourse.tile import TileContext

    f32 = mybir.dt.float32
    bf16 = mybir.dt.bfloat16
    D = EMBED_DIM

    nc = bacc.Bacc("TRN2", target_bir_lowering=False, debug=False,
                   enable_asserts=False, num_devices=N_CORES)

    xT = nc.dram_tensor("xT", [D, ROWS], bf16, kind="ExternalInput")
    wqT = nc.dram_tensor("wqT", [D, D], bf16, kind="ExternalInput")
    wkT = nc.dram_tensor("wkT", [D, D], bf16, kind="ExternalInput")
    wvT = nc.dram_tensor("wvT", [D, D], bf16, kind="ExternalInput")
    woT = nc.dram_tensor("woT", [D, D], bf16, kind="ExternalInput")
    cst = nc.dram_tensor("cst", [128, NCONST], f32, kind="ExternalInput")
    out = nc.dram_tensor("out", [D, TOK], f32, kind="ExternalOutput")

    with TileContext(nc) as tc:
        with tc.tile_pool(name="persist", bufs=1) as pers:
            # ---- input DMAs, ordered by first use
            xtb = pers.tile([128, KC * ROWS], bf16, name="xtb")
            nc.sync.dma_start(
                xtb[:].rearrange("p (c r) -> p c r", c=KC),
                xT.ap().rearrange("(c p) r -> p c r", p=128))
            wkt = pers.tile([128, KC * D], bf16, name="wkt")
            nc.sync.dma_start(
                wkt[:].rearrange("p (c n) -> p c n", c=KC),
                wkT.ap().rearrange("(c p) n -> p c n", p=128))
            wqt = pers.tile([128, KC * D], bf16, name="wqt")
            nc.sync.dma_start(
                wqt[:].rearrange("p (c n) -> p c n", c=KC),
                wqT.ap().rearrange("(c p) n -> p c n", p=128))
            cstt = pers.tile([128, NCONST], f32, name="cstt")
            nc.sync.dma_start(cstt[:], cst.ap())
            wvt = pers.tile([128, KC * D], bf16, name="wvt")
            nc.sync.dma_start(
                wvt[:].rearrange("p (c n) -> p c n", c=KC),
                wvT.ap().rearrange("(c p) n -> p c n", p=128))
            # wo: lhsT rows are ctx head dims (96 per head)
            wot = pers.tile([HEAD_DIM, NUM_HEADS * D], bf16, name="wot")
            nc.sync.dma_start(
                wot[:].rearrange("p (h n) -> p h n", h=NUM_HEADS),
                woT.ap().rearrange("(h p) n -> p h n", p=HEAD_DIM))

            bqt = cstt[0:HEAD_DIM, 0:NUM_HEADS]
            bo2t = cstt[:, 8:14]
            mkall = cstt[:, 14:NCONST].bitcast(bf16)   # [128, 384]
            masks = [mkall[:, 0:128], mkall[:, 128:320], mkall[0:64, 320:384]]

            qTh = [pers.tile([HEAD_DIM, TOK], bf16, name=f"qTh{h}")
                   for h in range(NUM_HEADS)]
            kTh = [pers.tile([HEAD_DIM, ROWS], bf16, name=f"kTh{h}")
                   for h in range(NUM_HEADS)]
            vt = [pers.tile([128, NUM_HEADS * VBLK], bf16, name=f"vt{r}")
                  for r in range(NVC)]
            for r in range(NVC):
                rows = min(128, ROWS - 128 * r)
                dst = vt[r][0:rows, :].rearrange("p (b c) -> p b c", c=VBLK)
                nc.gpsimd.memset(dst[:, :, HEAD_DIM:VBLK], 1.0)

            with tc.tile_pool(name="qkpsum", bufs=3, space="PSUM") as qkpsum, \
                 tc.tile_pool(name="vpsum", bufs=2, space="PSUM") as vpsum:
                # ---- k / q projections (weight-stationary, per-head M=96)
                for name, wt, dest, ncols, coff in (
                    ("k", wkt, kTh, ROWS, 0),
                    ("q", wqt, qTh, TOK, OVERLAP),
                ):
                    nsz = [512] * (ncols // 512) + \
                          ([ncols % 512] if ncols % 512 else [])
                    for h in range(NUM_HEADS):
                        n0 = 0
                        for sz in nsz:
                            ps = qkpsum.tile([HEAD_DIM, 512], f32, tag="pqk",
                                             name="ps_qk")
                            for c in range(KC):
                                nc.tensor.matmul(
                                    ps[:, 0:sz],
                                    wt[:, c * D + h * HEAD_DIM:
                                       c * D + (h + 1) * HEAD_DIM],
                                    xtb[:, c * ROWS + coff + n0:
                                        c * ROWS + coff + n0 + sz],
                                    start=(c == 0), stop=(c == KC - 1))
                            if name == "q":
                                nc.vector.tensor_scalar_add(
                                    dest[h][:, n0:n0 + sz], ps[:, 0:sz],
                                    bqt[:, h:h + 1])
                            else:
                                nc.vector.tensor_copy(
                                    dest[h][:, n0:n0 + sz], ps[:, 0:sz])
                            n0 += sz

                # ---- v projection (x-stationary, natural layout)
                for r in range(NVC):
                    rows = min(128, ROWS - 128 * r)
                    pv0 = vpsum.tile([128, 512], f32, tag="pv0", name="pv0")
                    pv1 = vpsum.tile([128, 256], f32, tag="pv1", name="pv1")
                    vps = [pv0, pv1]
                    for c in range(KC):
                        for i, (nn, sz) in enumerate(((0, 512), (512, 256))):
                            nc.tensor.matmul(
                                vps[i][0:rows, 0:sz],
                                xtb[:, c * ROWS + 128 * r:
                                    c * ROWS + 128 * r + rows],
                                wvt[:, c * D + nn: c * D + nn + sz],
                                start=(c == 0), stop=(c == KC - 1))
                    dst = vt[r][0:rows, :].rearrange("p (b c) -> p b c", c=VBLK)
                    nc.scalar.copy(
                        dst[:, 0:5, 0:HEAD_DIM],
                        pv0[0:rows, 0:5 * HEAD_DIM]
                        .rearrange("p (b c) -> p b c", c=HEAD_DIM))
                    # head 5 straddles the 512 boundary: cols 480:512 | 0:64
                    nc.scalar.copy(dst[:, 5, 0:32], pv0[0:rows, 480:512])
                    nc.scalar.copy(dst[:, 5, 32:HEAD_DIM], pv1[0:rows, 0:64])
                    nc.scalar.copy(
                        dst[:, 6:8, 0:HEAD_DIM],
                        pv1[0:rows, 64:64 + 2 * HEAD_DIM]
                        .rearrange("p (b c) -> p b c", c=HEAD_DIM))

            # ---- attention + normalization + out-projection, pipelined
            with tc.tile_pool(name="apool", bufs=2) as apool, \
                 tc.tile_pool(name="upool", bufs=1) as upool, \
                 tc.tile_pool(name="opool", bufs=2) as opool, \
                 tc.tile_pool(name="apsum", bufs=2, space="PSUM") as apsum, \
                 tc.tile_pool(name="opsum", bufs=2, space="PSUM") as opsum:
                ctxH = [upool.tile([HEAD_DIM, GRP], bf16, name=f"ctxH{g}_{h}",
                                   tag=f"ctxH{g}_{h}")
                        for g in range(NG) for h in range(NUM_HEADS)]

                def attention_head(g, h):
                    """Scores (windowed), exp+mask, ctx, and normalize."""
                    ctx_ps = apsum.tile([VBLK, GRP], f32, tag="ctx",
                                        name="ctx_ps")
                    for c, (k0, k1, w0, w1, mi) in enumerate(CHUNKS):
                        ksz = k1 - k0
                        win = w1 - w0
                        s_ps = apsum.tile([128, 192], f32, tag="s",
                                          name="s_ps", bufs=4)
                        nc.tensor.matmul(
                            s_ps[0:ksz, 0:win],
                            kTh[h][:, GRP * g + k0: GRP * g + k1],
                            qTh[h][:, GRP * g + w0: GRP * g + w1],
                            start=True, stop=True)
                        ex = apool.tile([128, 192], bf16, tag="ex", name="ex",
                                        bufs=6)
                        nc.scalar.activation(
                            ex[0:ksz, 0:win], s_ps[0:ksz, 0:win],
                            mybir.ActivationFunctionType.Exp)
                        nc.vector.tensor_tensor(
                            out=ex[0:ksz, 0:win], in0=ex[0:ksz, 0:win],
                            in1=masks[mi][0:ksz, 0:win],
                            op=mybir.AluOpType.mult)
                        nc.tensor.matmul(
                            ctx_ps[:, w0:w1],
                            vt[4 * g + c][0:ksz, h * VBLK:(h + 1) * VBLK],
                            ex[0:ksz, 0:win],
                            start=(c == 0), stop=(c == len(CHUNKS) - 1),
                            skip_group_check=True)
                    # normalize: 1/d via exp(-ln d) on ACT, broadcast on
                    # GPSIMD, multiply on DVE — all off the PE critical path
                    rl = apool.tile([1, GRP], f32, tag="rl", name="rl", bufs=2)
                    nc.scalar.activation(rl[:], ctx_ps[HEAD_DIM:VBLK, :],
                                         mybir.ActivationFunctionType.Ln)
                    nc.scalar.activation(rl[:], rl[:],
                                         mybir.ActivationFunctionType.Exp,
                                         scale=-1.0)
                    rdb = apool.tile([HEAD_DIM, GRP], f32, tag="rdb",
                                     name="rdb", bufs=2)
                    nc.gpsimd.partition_broadcast(rdb[:], rl[:])
                    nc.vector.tensor_tensor(
                        out=ctxH[g * NUM_HEADS + h][:],
                        in0=ctx_ps[0:HEAD_DIM, :], in1=rdb[:],
                        op=mybir.AluOpType.mult)

                def outproj_half(i):
                    ostT = opool.tile([128, KC * 512], f32, tag="ost",
                                      name="ostT")
                    for dc in range(KC):
                        op = opsum.tile([128, 512], f32, tag="po", name="ps_o")
                        for h in range(NUM_HEADS):
                            nc.tensor.matmul(
                                op[:],
                                wot[:, h * D + dc * 128: h * D + dc * 128 + 128],
                                ctxH[i * NUM_HEADS + h][:],
                                start=(h == 0), stop=(h == NUM_HEADS - 1))
                        nc.vector.tensor_scalar_add(
                            ostT[:, dc * 512:(dc + 1) * 512], op[:],
                            bo2t[:, dc:dc + 1])
                    nc.sync.dma_start(
                        out.ap()[:, 512 * i: 512 * (i + 1)]
                        .rearrange("(c p) n -> p c n", p=128),
                        ostT[:].rearrange("p (c n) -> p c n", c=KC))

                for g in range(NG):
                    for h in range(NUM_HEADS):
                        attention_head(g, h)
                for i in range(NG):
                    outproj_half(i)
    nc.compile()
    return nc


_program_cache = {}


def _get_program():
    if "nc" not in _program_cache:
        _program_cache["nc"] = _build_program()
    return _program_cache["nc"]


def _host_masks():
    # Three mask patterns: d = key - token offset within the chunk window.
    # m0 (first chunk): d = kk - mm; m1/m2 (later chunks): d = kk - mm + 64.
    import ml_dtypes
    masks = []
    for (nk, nw, off) in ((128, 128, 0), (128, 192, HALO), (64, 64, HALO)):
        kk, mm = np.meshgrid(np.arange(nk), np.arange(nw), indexing="ij")
        d = kk - mm + off
        valid = (d >= 0) & (d <= HALO) & (d % 4 == 0) & (d != OVERLAP)
        masks.append(valid.astype(ml_dtypes.bfloat16))
    return masks


def kernel(main, begin, end, in_proj_w, in_proj_b, out_proj_w, out_proj_b):
    import ml_dtypes
    from concourse.bass_utils import run_bass_kernel_spmd

    bf = ml_dtypes.bfloat16
    main = np.asarray(main, np.float32)
    begin = np.asarray(begin, np.float32)
    end = np.asarray(end, np.float32)
    in_proj_w = np.asarray(in_proj_w, np.float32)
    in_proj_b = np.asarray(in_proj_b, np.float32)
    out_proj_w = np.asarray(out_proj_w, np.float32)
    out_proj_b = np.asarray(out_proj_b, np.float32)

    D = EMBED_DIM
    scale = HEAD_DIM ** -0.5
    wq, wk, wv = in_proj_w[:D], in_proj_w[D:2 * D], in_proj_w[2 * D:]
    bq_, bv = in_proj_b[:D], in_proj_b[2 * D:3 * D]
    combined = np.concatenate([begin, main, end], axis=0)  # [N + 64, D]

    wqT = np.ascontiguousarray(wq.T * scale).astype(bf)
    wkT = np.ascontiguousarray(wk.T).astype(bf)
    wvT = np.ascontiguousarray(wv.T).astype(bf)
    woT = np.ascontiguousarray(out_proj_w.T).astype(bf)

    cst = np.zeros((128, NCONST), np.float32)
    cst[0:HEAD_DIM, 0:NUM_HEADS] = (bq_ * scale).reshape(NUM_HEADS, HEAD_DIM).T
    bo2 = out_proj_w @ bv + out_proj_b                      # [768]
    cst[:, 8:14] = bo2.reshape(KC, 128).T
    masks = _host_masks()
    mk = cst[:, 14:NCONST].view(bf)                         # [128, 384]
    mk[:, 0:128] = masks[0]
    mk[:, 128:320] = masks[1]
    mk[0:64, 320:384] = masks[2]

    shared = {"wqT": wqT, "wkT": wkT, "wvT": wvT, "woT": woT, "cst": cst}
    in_maps = []
    for c in range(N_CORES):
        xTc = np.ascontiguousarray(
            combined[c * TOK: c * TOK + ROWS].T).astype(bf)
        in_maps.append({**shared, "xT": xTc})

    nc = _get_program()
    res = run_bass_kernel_spmd(nc, in_maps, core_ids=list(range(N_CORES)),
                               **_program_cache.get("run_kwargs", {}))
    _program_cache["last_result"] = res

    outp = np.empty((N_LINES, 2 * D), np.float32)
    outp[:, :D] = main
    for c in range(N_CORES):
        outp[c * TOK:(c + 1) * TOK, D:] = res.results[c]["out"].T
    return outp


# revision 5
# speedup vs baseline: 1.3558x; 1.3558x over previous
"""Trainium2 Bass kernel for dilated sliding-window attention (AttnWrapper).

Reference computation (all fp32):
  combined = [begin | main | end]                       # [8256, 768]
  keys[t]  = combined[t + 32 + off], off in +-{4..32}   # 16 dilated window keys
  q = (main @ wq.T + bq) * 96**-0.5
  k/v = keys @ w{k,v}.T + b{k,v}
  attn = softmax(q.k), ctx = attn.v, out = [main | ctx @ wo.T + bo]

Sharding: tokens across 8 cores (1024 each) with a 64-row halo of the
combined buffer; weights replicated. Each core computes attn_outT
[768, 1024]; the host transposes and concatenates with main.

Device-side notes (v2 — all-bf16 matmul pipeline):
 - All matmul operands bf16 (cast on host). bf16 runs 1 cycle/col at any
   size on PE (f32r needs >=256 cols), so score matmuls stream only each
   key-chunk's valid token window (128/192/192/192/64) instead of 512.
 - bk dropped (softmax shift invariance); bv folded into bo2 = wo@bv + bo;
   q scale and bq folded into wq/bq on the host.
 - A ones-column per V head block makes the ctx matmul also produce the
   softmax denominator (row 96 of the ctx PSUM tile). 1/d runs on DVE
   (reciprocal_approx_fast — keeps ACT on the Exp table, avoiding
   ~1.3us table reloads per Ln/Exp switch), broadcast on GPSIMD,
   multiply on DVE; all off the PE critical path.
 - Phases: v-proj, k-proj, then q-proj software-pipelined with group-0
   attention (head h's attention is emitted during head h+1's q chains),
   then group-1 attention interleaved with group-0 out-projection.
 - Per-chunk input DMAs ordered so the first v matmul starts ~8us in.
"""

import numpy as np

EMBED_DIM = 768
NUM_HEADS = 8
HEAD_DIM = 96
OVERLAP = 32
HALO = 2 * OVERLAP          # 64 extra combined rows per core
N_LINES = 8192
N_CORES = 8
TOK = N_LINES // N_CORES    # 1024 tokens per core
ROWS = TOK + HALO           # 1088 combined rows per core
GRP = 512                   # tokens per attention group
NG = TOK // GRP             # 2 groups
# key chunks (start, end), valid token windows (w0, w1), mask index
CHUNKS = [(0, 128, 0, 128, 0), (128, 256, 64, 256, 1), (256, 384, 192, 384, 1),
          (384, 512, 320, 512, 1), (512, 576, 448, 512, 2)]
VBLK = HEAD_DIM + 1         # 97: v head block + ones column
KC = EMBED_DIM // 128       # 6 contraction chunks of 128
NVC = (ROWS + 127) // 128   # 9 v row-chunks (8x128 + 64)
NCONST = 8 + 6 + 192        # bq | bo2 | masks(bf16 pairs)


def _build_program():
    import concourse.bacc as bacc
    import concourse.mybir as mybir
    from concourse.tile import TileContext

    f32 = mybir.dt.float32
    bf16 = mybir.dt.bfloat16
    D = EMBED_DIM

    nc = bacc.Bacc("TRN2", target_bir_lowering=False, debug=False,
                   enable_asserts=False, num_devices=N_CORES)

    xT = nc.dram_tensor("xT", [D, ROWS], bf16, kind="ExternalInput")
    wqT = nc.dram_tensor("wqT", [D, D], bf16, kind="ExternalInput")
    wkT = nc.dram_tensor("wkT", [D, D], bf16, kind="ExternalInput")
    wvT = nc.dram_tensor("wvT", [D, D], bf16, kind="ExternalInput")
    woT = nc.dram_tensor("woT", [D, D], bf16, kind="ExternalInput")
    cst = nc.dram_tensor("cst", [128, NCONST], f32, kind="ExternalInput")
    out = nc.dram_tensor("out", [D, TOK], f32, kind="ExternalOutput")

    with TileContext(nc) as tc:
        with tc.tile_pool(name="persist", bufs=1) as pers:
            vtile = [pers.tile([128, NUM_HEADS * VBLK], bf16, name=f"vt{r}")
                     for r in range(NVC)]
            for r in range(NVC):
                rows = min(128, ROWS - 128 * r)
                dst = vtile[r][0:rows, :].rearrange("p (b c) -> p b c", c=VBLK)
                nc.gpsimd.memset(dst[:, :, HEAD_DIM:VBLK], 1.0)

            # ---- input DMAs: per-chunk tiles, ordered by first use
            xc = [pers.tile([128, ROWS], bf16, name=f"xc{c}")
                  for c in range(KC)]
            wvc = [pers.tile([128, D], bf16, name=f"wvc{c}")
                   for c in range(KC)]
            wkc = [pers.tile([128, D], bf16, name=f"wkc{c}")
                   for c in range(KC)]
            wqc = [pers.tile([128, D], bf16, name=f"wqc{c}")
                   for c in range(KC)]
            for c in range(KC):
                nc.sync.dma_start(xc[c][:], xT.ap()[c * 128:(c + 1) * 128, :])
                nc.sync.dma_start(wvc[c][:], wvT.ap()[c * 128:(c + 1) * 128, :])
            cstt = pers.tile([128, NCONST], f32, name="cstt")
            nc.sync.dma_start(cstt[:], cst.ap())
            for c in range(KC):
                nc.sync.dma_start(wkc[c][:], wkT.ap()[c * 128:(c + 1) * 128, :])
            for c in range(KC):
                nc.sync.dma_start(wqc[c][:], wqT.ap()[c * 128:(c + 1) * 128, :])
            wot = pers.tile([HEAD_DIM, NUM_HEADS * D], bf16, name="wot")
            nc.sync.dma_start(
                wot[:].rearrange("p (h n) -> p h n", h=NUM_HEADS),
                woT.ap().rearrange("(h p) n -> p h n", p=HEAD_DIM))

            bqt = cstt[0:HEAD_DIM, 0:NUM_HEADS]
            bo2t = cstt[:, 8:14]
            mkall = cstt[:, 14:NCONST].bitcast(bf16)   # [128, 384]
            masks = [mkall[:, 0:128], mkall[:, 128:320], mkall[0:64, 320:384]]

            qTh = [pers.tile([HEAD_DIM, TOK], bf16, name=f"qTh{h}")
                   for h in range(NUM_HEADS)]
            kTh = [pers.tile([HEAD_DIM, ROWS], bf16, name=f"kTh{h}")
                   for h in range(NUM_HEADS)]

            # ---- v projection (x-stationary, natural), then k projection
            with tc.tile_pool(name="kpsum", bufs=3, space="PSUM") as kpsum:
                with tc.tile_pool(name="vpsum", bufs=2, space="PSUM") as vps:
                    for r in range(NVC):
                        rows = min(128, ROWS - 128 * r)
                        pv0 = vps.tile([128, 512], f32, tag="pv0", name="pv0")
                        pv1 = vps.tile([128, 256], f32, tag="pv1", name="pv1")
                        pv = [pv0, pv1]
                        for c in range(KC):
                            for i, (nn, sz) in enumerate(((0, 512), (512, 256))):
                                nc.tensor.matmul(
                                    pv[i][0:rows, 0:sz],
                                    xc[c][:, 128 * r: 128 * r + rows],
                                    wvc[c][:, nn: nn + sz],
                                    start=(c == 0), stop=(c == KC - 1))
                        dst = vtile[r][0:rows, :].rearrange(
                            "p (b c) -> p b c", c=VBLK)
                        nc.scalar.copy(
                            dst[:, 0:5, 0:HEAD_DIM],
                            pv0[0:rows, 0:5 * HEAD_DIM]
                            .rearrange("p (b c) -> p b c", c=HEAD_DIM))
                        # head 5 straddles the 512 boundary: 480:512 | 0:64
                        nc.scalar.copy(dst[:, 5, 0:32], pv0[0:rows, 480:512])
                        nc.scalar.copy(dst[:, 5, 32:HEAD_DIM],
                                       pv1[0:rows, 0:64])
                        nc.scalar.copy(
                            dst[:, 6:8, 0:HEAD_DIM],
                            pv1[0:rows, 64:64 + 2 * HEAD_DIM]
                            .rearrange("p (b c) -> p b c", c=HEAD_DIM))

                # k projection (weight-stationary, per-head M=96)
                for h in range(NUM_HEADS):
                    for n0, sz in ((0, 512), (512, 512), (1024, 64)):
                        ps = kpsum.tile([HEAD_DIM, 512], f32, tag="pqk",
                                        name="ps_k")
                        for c in range(KC):
                            nc.tensor.matmul(
                                ps[:, 0:sz],
                                wkc[c][:, h * HEAD_DIM:(h + 1) * HEAD_DIM],
                                xc[c][:, n0: n0 + sz],
                                start=(c == 0), stop=(c == KC - 1))
                        nc.vector.tensor_copy(kTh[h][:, n0:n0 + sz],
                                              ps[:, 0:sz])

            # ---- q projection pipelined with attention + out-projection
            with tc.tile_pool(name="apool", bufs=2) as apool, \
                 tc.tile_pool(name="upool", bufs=1) as upool, \
                 tc.tile_pool(name="opool", bufs=2) as opool, \
                 tc.tile_pool(name="mpsum", bufs=2, space="PSUM") as mpsum, \
                 tc.tile_pool(name="apsum", bufs=2, space="PSUM") as apsum, \
                 tc.tile_pool(name="spsum", bufs=2, space="PSUM") as spsum, \
                 tc.tile_pool(name="opsum", bufs=2, space="PSUM") as opsum:
                ctxH = [upool.tile([HEAD_DIM, GRP], bf16, name=f"ctxH{g}_{h}",
                                   tag=f"ctxH{g}_{h}")
                        for g in range(NG) for h in range(NUM_HEADS)]

                def q_head(h):
                    for n0 in (0, 512):
                        ps = mpsum.tile([HEAD_DIM, 512], f32, tag="pqk",
                                        name="ps_q")
                        for c in range(KC):
                            nc.tensor.matmul(
                                ps[:],
                                wqc[c][:, h * HEAD_DIM:(h + 1) * HEAD_DIM],
                                xc[c][:, OVERLAP + n0: OVERLAP + n0 + 512],
                                start=(c == 0), stop=(c == KC - 1))
                        nc.vector.tensor_scalar_add(
                            qTh[h][:, n0:n0 + 512], ps[:], bqt[:, h:h + 1])

                def attention_head(g, h):
                    """Scores (windowed), exp+mask, ctx, and normalize."""
                    ctx_ps = apsum.tile([VBLK, GRP], f32, tag="ctx",
                                        name="ctx_ps")
                    for c, (k0, k1, w0, w1, mi) in enumerate(CHUNKS):
                        ksz = k1 - k0
                        win = w1 - w0
                        s_ps = spsum.tile([128, 192], f32, tag="s",
                                          name="s_ps")
                        nc.tensor.matmul(
                            s_ps[0:ksz, 0:win],
                            kTh[h][:, GRP * g + k0: GRP * g + k1],
                            qTh[h][:, GRP * g + w0: GRP * g + w1],
                            start=True, stop=True)
                        ex = apool.tile([128, 192], bf16, tag="ex", name="ex",
                                        bufs=6)
                        nc.scalar.activation(
                            ex[0:ksz, 0:win], s_ps[0:ksz, 0:win],
                            mybir.ActivationFunctionType.Exp)
                        nc.vector.tensor_tensor(
                            out=ex[0:ksz, 0:win], in0=ex[0:ksz, 0:win],
                            in1=masks[mi][0:ksz, 0:win],
                            op=mybir.AluOpType.mult)
                        nc.tensor.matmul(
                            ctx_ps[:, w0:w1],
                            vtile[4 * g + c][0:ksz, h * VBLK:(h + 1) * VBLK],
                            ex[0:ksz, 0:win],
                            start=(c == 0), stop=(c == len(CHUNKS) - 1),
                            skip_group_check=True)
    # 1/d on DVE (no ACT table switch), broadcast on GPSIMD.
                    # The d row goes through SBUF: the custom DVE op does
                    # not read PSUM at a partition offset correctly.
                    rl0 = apool.tile([1, GRP], f32, tag="rl0", name="rl0",
                                     bufs=2)
                    nc.vector.tensor_copy(rl0[:], ctx_ps[HEAD_DIM:VBLK, :])
                    rl = apool.tile([1, GRP], f32, tag="rl", name="rl", bufs=2)
                    nc.vector.reciprocal_approx_fast(rl[:], rl0[:])
                    rdb = apool.tile([HEAD_DIM, GRP], f32, tag="rdb",
                                     name="rdb", bufs=2)
                    nc.gpsimd.partition_broadcast(rdb[:], rl[:])
                    nc.vector.tensor_tensor(
                        out=ctxH[g * NUM_HEADS + h][:],
                        in0=ctx_ps[0:HEAD_DIM, :], in1=rdb[:],
                        op=mybir.AluOpType.mult)

                def outproj_dc(i, dc):
                    op = opsum.tile([128, 512], f32, tag="po", name="ps_o")
                    for h in range(NUM_HEADS):
                        nc.tensor.matmul(
                            op[:],
                            wot[:, h * D + dc * 128: h * D + dc * 128 + 128],
                            ctxH[i * NUM_HEADS + h][:],
                            start=(h == 0), stop=(h == NUM_HEADS - 1))
                    ost = opool.tile([128, 512], f32, tag="ost", name="ost")
                    nc.vector.tensor_scalar_add(ost[:], op[:],
                                                bo2t[:, dc:dc + 1])
                    nc.sync.dma_start(
                        out.ap()[dc * 128:(dc + 1) * 128,
                                 512 * i: 512 * (i + 1)], ost[:])

                # q chains one head ahead of group-0 attention
                for h in range(NUM_HEADS + 1):
                    if h < NUM_HEADS:
                        q_head(h)
                    if h > 0:
                        attention_head(0, h - 1)
                # group-1 attention interleaved with group-0 out-projection
                for h in range(NUM_HEADS):
                    attention_head(1, h)
                    if h % 2 == 1:
                        dc = (h - 1) // 2
                        outproj_dc(0, dc)
                for dc in range(4, KC):
                    outproj_dc(0, dc)
                for dc in range(KC):
                    outproj_dc(1, dc)
    nc.compile()
    return nc


_program_cache = {}


def _get_program():
    if "nc" not in _program_cache:
        _program_cache["nc"] = _build_program()
    return _program_cache["nc"]


def _host_masks():
    # Three mask patterns: d = key - token offset within the chunk window.
    # m0 (first chunk): d = kk - mm; m1/m2 (later chunks): d = kk - mm + 64.
    import ml_dtypes
    masks = []
    for (nk, nw, off) in ((128, 128, 0), (128, 192, HALO), (64, 64, HALO)):
        kk, mm = np.meshgrid(np.arange(nk), np.arange(nw), indexing="ij")
        d = kk - mm + off
        valid = (d >= 0) & (d <= HALO) & (d % 4 == 0) & (d != OVERLAP)
        masks.append(valid.astype(ml_dtypes.bfloat16))
    return masks


def kernel(main, begin, end, in_proj_w, in_proj_b, out_proj_w, out_proj_b):
    import ml_dtypes
    from concourse.bass_utils import run_bass_kernel_spmd

    bf = ml_dtypes.bfloat16
    main = np.asarray(main, np.float32)
    begin = np.asarray(begin, np.float32)
    end = np.asarray(end, np.float32)
    in_proj_w = np.asarray(in_proj_w, np.float32)
    in_proj_b = np.asarray(in_proj_b, np.float32)
    out_proj_w = np.asarray(out_proj_w, np.float32)
    out_proj_b = np.asarray(out_proj_b, np.float32)

    D = EMBED_DIM
    scale = HEAD_DIM ** -0.5
    wq, wk, wv = in_proj_w[:D], in_proj_w[D:2 * D], in_proj_w[2 * D:]
    bq_, bv = in_proj_b[:D], in_proj_b[2 * D:3 * D]
    combined = np.concatenate([begin, main, end], axis=0)  # [N + 64, D]

    wqT = np.ascontiguousarray(wq.T * scale).astype(bf)
    wkT = np.ascontiguousarray(wk.T).astype(bf)
    wvT = np.ascontiguousarray(wv.T).astype(bf)
    woT = np.ascontiguousarray(out_proj_w.T).astype(bf)

    cst = np.zeros((128, NCONST), np.float32)
    cst[0:HEAD_DIM, 0:NUM_HEADS] = (bq_ * scale).reshape(NUM_HEADS, HEAD_DIM).T
    bo2 = out_proj_w @ bv + out_proj_b                      # [768]
    cst[:, 8:14] = bo2.reshape(KC, 128).T
    masks = _host_masks()
    mk = cst[:, 14:NCONST].view(bf)                         # [128, 384]
    mk[:, 0:128] = masks[0]
    mk[:, 128:320] = masks[1]
    mk[0:64, 320:384] = masks[2]

    shared = {"wqT": wqT, "wkT": wkT, "wvT": wvT, "woT": woT, "cst": cst}
    in_maps = []
    for c in range(N_CORES):
        xTc = np.ascontiguousarray(
            combined[c * TOK: c * TOK + ROWS].T).astype(bf)
        in_maps.append({**shared, "xT": xTc})

    nc = _get_program()
    res = run_bass_kernel_spmd(nc, in_maps, core_ids=list(range(N_CORES)),
                               **_program_cache.get("run_kwargs", {}))
    _program_cache["last_result"] = res

    outp = np.empty((N_LINES, 2 * D), np.float32)
    outp[:, :D] = main
    for c in range(N_CORES):
        outp[c * TOK:(c + 1) * TOK, D:] = res.results[c]["out"].T
    return outp


# revision 10
# speedup vs baseline: 1.3988x; 1.0317x over previous
"""Trainium2 Bass kernel for dilated sliding-window attention (AttnWrapper).

Reference computation (all fp32):
  combined = [begin | main | end]                       # [8256, 768]
  keys[t]  = combined[t + 32 + off], off in +-{4..32}   # 16 dilated window keys
  q = (main @ wq.T + bq) * 96**-0.5
  k/v = keys @ w{k,v}.T + b{k,v}
  attn = softmax(q.k), ctx = attn.v, out = [main | ctx @ wo.T + bo]

Sharding: tokens across 8 cores (1024 each) with a 64-row halo of the
combined buffer; weights replicated. Each core computes attn_outT
[768, 1024]; the host transposes and concatenates with main.

Device-side notes (v2 — all-bf16 matmul pipeline):
 - All matmul operands bf16 (cast on host). bf16 runs 1 cycle/col at any
   size on PE (f32r needs >=256 cols), so score matmuls stream only each
   key-chunk's valid token window (128/192/192/192/64) instead of 512.
 - bk dropped (softmax shift invariance); bv folded into bo2 = wo@bv + bo;
   q scale and bq folded into wq/bq on the host.
 - A ones-column per V head block makes the ctx matmul also produce the
   softmax denominator (row 96 of the ctx PSUM tile). 1/d runs on DVE
   (reciprocal_approx_fast — keeps ACT on the Exp table, avoiding
   ~1.3us table reloads per Ln/Exp switch), broadcast on GPSIMD,
   multiply on DVE; all off the PE critical path.
 - Phases: v-proj, k-proj, then q-proj software-pipelined with group-0
   attention (head h's attention is emitted during head h+1's q chains),
   then group-1 attention interleaved with group-0 out-projection.
 - Per-chunk input DMAs ordered so the first v matmul starts ~8us in.
"""

import numpy as np

EMBED_DIM = 768
NUM_HEADS = 8
HEAD_DIM = 96
OVERLAP = 32
HALO = 2 * OVERLAP          # 64 extra combined rows per core
N_LINES = 8192
N_CORES = 8
TOK = N_LINES // N_CORES    # 1024 tokens per core
ROWS = TOK + HALO           # 1088 combined rows per core
GRP = 512                   # tokens per attention group
NG = TOK // GRP             # 2 groups
# key chunks (start, end), valid token windows (w0, w1), mask index
CHUNKS = [(0, 128, 0, 128, 0), (128, 256, 64, 256, 1), (256, 384, 192, 384, 1),
          (384, 512, 320, 512, 1), (512, 576, 448, 512, 2)]
VBLK = HEAD_DIM + 1         # 97: v head block + ones column
KC = EMBED_DIM // 128       # 6 contraction chunks of 128
NVC = (ROWS + 127) // 128   # 9 v row-chunks (8x128 + 64)
NCONST = 8 + 6 + 192        # bq | bo2 | masks(bf16 pairs)


def _build_program():
    import concourse.bacc as bacc
    import concourse.mybir as mybir
    from concourse.tile import TileContext

    f32 = mybir.dt.float32
    bf16 = mybir.dt.bfloat16
    D = EMBED_DIM

    nc = bacc.Bacc("TRN2", target_bir_lowering=False, debug=False,
                   enable_asserts=False, num_devices=N_CORES)

    fp8 = mybir.dt.float8e4
    PM = mybir.MatmulPerfMode.DoubleRow

    xT = nc.dram_tensor("xT", [D, ROWS], bf16, kind="ExternalInput")
    x8T = nc.dram_tensor("x8T", [D, ROWS], fp8, kind="ExternalInput")
    wqT = nc.dram_tensor("wqT", [D, D], bf16, kind="ExternalInput")
    wkT = nc.dram_tensor("wkT", [D, D], bf16, kind="ExternalInput")
    wv8T = nc.dram_tensor("wv8T", [D, D], fp8, kind="ExternalInput")
    woT = nc.dram_tensor("woT", [D, D], bf16, kind="ExternalInput")
    cst = nc.dram_tensor("cst", [128, NCONST], f32, kind="ExternalInput")
    out = nc.dram_tensor("out", [D, TOK], f32, kind="ExternalOutput")

    with TileContext(nc) as tc:
        with tc.tile_pool(name="persist", bufs=1) as pers:
            vtile = [pers.tile([128, NUM_HEADS * VBLK], bf16, name=f"vt{r}")
                     for r in range(NVC)]
            for r in range(NVC):
                rows = min(128, ROWS - 128 * r)
                dst = vtile[r][0:rows, :].rearrange("p (b c) -> p b c", c=VBLK)
                nc.gpsimd.memset(dst[:, :, HEAD_DIM:VBLK], 1.0)

            # ---- input DMAs: per-chunk tiles, ordered by first use.
            # v runs first in fp8 DoubleRow: x/wv land as [128, 2, n] pairs.
            xp = [pers.tile([128, 2 * ROWS], fp8, name=f"xp{q}")
                  for q in range(KC // 2)]
            wvp = [pers.tile([128, 2 * D], fp8, name=f"wvp{q}")
                   for q in range(KC // 2)]
            xc = [pers.tile([128, ROWS], bf16, name=f"xc{c}")
                  for c in range(KC)]
            wkc = [pers.tile([128, D], bf16, name=f"wkc{c}")
                   for c in range(KC)]
            wqc = [pers.tile([128, D], bf16, name=f"wqc{c}")
                   for c in range(KC)]
            for q in range(KC // 2):
                nc.sync.dma_start(
                    xp[q][:].rearrange("p (k r) -> p k r", k=2),
                    x8T.ap()[256 * q:256 * (q + 1), :]
                    .rearrange("(k p) r -> p k r", p=128))
                nc.sync.dma_start(
                    wvp[q][:].rearrange("p (k n) -> p k n", k=2),
                    wv8T.ap()[256 * q:256 * (q + 1), :]
                    .rearrange("(k p) n -> p k n", p=128))
            cstt = pers.tile([128, NCONST], f32, name="cstt")
            nc.sync.dma_start(cstt[:], cst.ap())
            for c in range(KC):
                nc.sync.dma_start(xc[c][:], xT.ap()[c * 128:(c + 1) * 128, :])
                nc.sync.dma_start(wkc[c][:], wkT.ap()[c * 128:(c + 1) * 128, :])
            for c in range(KC):
                nc.sync.dma_start(wqc[c][:], wqT.ap()[c * 128:(c + 1) * 128, :])
            wot = pers.tile([HEAD_DIM, NUM_HEADS * D], bf16, name="wot")
            nc.sync.dma_start(
                wot[:].rearrange("p (h n) -> p h n", h=NUM_HEADS),
                woT.ap().rearrange("(h p) n -> p h n", p=HEAD_DIM))

            bqt = cstt[0:HEAD_DIM, 0:NUM_HEADS]
            bo2t = cstt[:, 8:14]
            mkall = cstt[:, 14:NCONST].bitcast(bf16)   # [128, 384]
            masks = [mkall[:, 0:128], mkall[:, 128:320], mkall[0:64, 320:384]]

            qTh = [pers.tile([HEAD_DIM, TOK], bf16, name=f"qTh{h}")
                   for h in range(NUM_HEADS)]
            kTh = [pers.tile([HEAD_DIM, ROWS], bf16, name=f"kTh{h}")
                   for h in range(NUM_HEADS)]

            # ---- v projection (x-stationary, natural), then k projection
            with tc.tile_pool(name="kpsum", bufs=3, space="PSUM") as kpsum:
                with tc.tile_pool(name="vpsum", bufs=2, space="PSUM") as vps:
                    Copy = mybir.ActivationFunctionType.Copy
                    for r in range(NVC):
                        rows = min(128, ROWS - 128 * r)
                        pv0 = vps.tile([128, 512], f32, tag="pv0", name="pv0")
                        pv1 = vps.tile([128, 256], f32, tag="pv1", name="pv1")
                        pv = [pv0, pv1]
                        for q in range(KC // 2):
                            for i, (nn, sz) in enumerate(((0, 512), (512, 256))):
                                nc.tensor.matmul(
                                    pv[i][0:rows, 0:sz],
                                    xp[q][:].rearrange("p (k r) -> p k r", k=2)
                                    [:, :, 128 * r: 128 * r + rows],
                                    wvp[q][:].rearrange("p (k n) -> p k n", k=2)
                                    [:, :, nn: nn + sz],
                                    start=(q == 0), stop=(q == KC // 2 - 1),
                                    perf_mode=PM)
                        # wv is host-scaled by 64 (e4m3 normal range);
                        # descale on the PSUM->SBUF copies
                        dst = vtile[r][0:rows, :].rearrange(
                            "p (b c) -> p b c", c=VBLK)
                        nc.scalar.activation(
                            dst[:, 0:5, 0:HEAD_DIM],
                            pv0[0:rows, 0:5 * HEAD_DIM]
                            .rearrange("p (b c) -> p b c", c=HEAD_DIM),
                            Copy, scale=1.0 / 64)
                        # head 5 straddles the 512 boundary: 480:512 | 0:64
                        nc.scalar.activation(dst[:, 5, 0:32],
                                             pv0[0:rows, 480:512],
                                             Copy, scale=1.0 / 64)
                        nc.scalar.activation(dst[:, 5, 32:HEAD_DIM],
                                             pv1[0:rows, 0:64],
                                             Copy, scale=1.0 / 64)
                        nc.scalar.activation(
                            dst[:, 6:8, 0:HEAD_DIM],
                            pv1[0:rows, 64:64 + 2 * HEAD_DIM]
                            .rearrange("p (b c) -> p b c", c=HEAD_DIM),
                            Copy, scale=1.0 / 64)

                # k projection (weight-stationary, per-head M=96)
                for h in range(NUM_HEADS):
                    for n0, sz in ((0, 512), (512, 512), (1024, 64)):
                        ps = kpsum.tile([HEAD_DIM, 512], f32, tag="pqk",
                                        name="ps_k")
                        for c in range(KC):
                            nc.tensor.matmul(
                                ps[:, 0:sz],
                                wkc[c][:, h * HEAD_DIM:(h + 1) * HEAD_DIM],
                                xc[c][:, n0: n0 + sz],
                                start=(c == 0), stop=(c == KC - 1))
                        nc.vector.tensor_copy(kTh[h][:, n0:n0 + sz],
                                              ps[:, 0:sz])

            # ---- q projection pipelined with attention + out-projection
            with tc.tile_pool(name="apool", bufs=2) as apool, \
                 tc.tile_pool(name="upool", bufs=1) as upool, \
                 tc.tile_pool(name="opool", bufs=2) as opool, \
                 tc.tile_pool(name="mpsum", bufs=2, space="PSUM") as mpsum, \
                 tc.tile_pool(name="apsum", bufs=2, space="PSUM") as apsum, \
                 tc.tile_pool(name="spsum", bufs=2, space="PSUM") as spsum, \
                 tc.tile_pool(name="opsum", bufs=2, space="PSUM") as opsum:
                ctxH = [upool.tile([HEAD_DIM, GRP], bf16, name=f"ctxH{g}_{h}",
                                   tag=f"ctxH{g}_{h}")
                        for g in range(NG) for h in range(NUM_HEADS)]

                def q_head(h):
                    for n0 in (0, 512):
                        ps = mpsum.tile([HEAD_DIM, 512], f32, tag="pqk",
                                        name="ps_q")
                        for c in range(KC):
                            nc.tensor.matmul(
                                ps[:],
                                wqc[c][:, h * HEAD_DIM:(h + 1) * HEAD_DIM],
                                xc[c][:, OVERLAP + n0: OVERLAP + n0 + 512],
                                start=(c == 0), stop=(c == KC - 1))
                        nc.vector.tensor_scalar_add(
                            qTh[h][:, n0:n0 + 512], ps[:], bqt[:, h:h + 1])

                def attention_head(g, h):
                    """Scores (windowed), exp+mask, ctx, and normalize."""
                    ctx_ps = apsum.tile([VBLK, GRP], f32, tag="ctx",
                                        name="ctx_ps")
                    for c, (k0, k1, w0, w1, mi) in enumerate(CHUNKS):
                        ksz = k1 - k0
                        win = w1 - w0
                        s_ps = spsum.tile([128, 192], f32, tag="s",
                                          name="s_ps")
                        nc.tensor.matmul(
                            s_ps[0:ksz, 0:win],
                            kTh[h][:, GRP * g + k0: GRP * g + k1],
                            qTh[h][:, GRP * g + w0: GRP * g + w1],
                            start=True, stop=True)
                        ex = apool.tile([128, 192], bf16, tag="ex", name="ex",
                                        bufs=6)
                        nc.scalar.activation(
                            ex[0:ksz, 0:win], s_ps[0:ksz, 0:win],
                            mybir.ActivationFunctionType.Exp)
                        nc.vector.tensor_tensor(
                            out=ex[0:ksz, 0:win], in0=ex[0:ksz, 0:win],
                            in1=masks[mi][0:ksz, 0:win],
                            op=mybir.AluOpType.mult)
                        nc.tensor.matmul(
                            ctx_ps[:, w0:w1],
                            vtile[4 * g + c][0:ksz, h * VBLK:(h + 1) * VBLK],
                            ex[0:ksz, 0:win],
                            start=(c == 0), stop=(c == len(CHUNKS) - 1),
                            skip_group_check=True)
    # 1/d on DVE (no ACT table switch), broadcast on GPSIMD.
                    # The d row goes through SBUF: the custom DVE op does
                    # not read PSUM at a partition offset correctly.
                    rl0 = apool.tile([1, GRP], f32, tag="rl0", name="rl0",
                                     bufs=2)
                    nc.vector.tensor_copy(rl0[:], ctx_ps[HEAD_DIM:VBLK, :])
                    rl = apool.tile([1, GRP], f32, tag="rl", name="rl", bufs=2)
                    nc.vector.reciprocal_approx_fast(rl[:], rl0[:])
                    rdb = apool.tile([HEAD_DIM, GRP], f32, tag="rdb",
                                     name="rdb", bufs=2)
                    nc.gpsimd.partition_broadcast(rdb[:], rl[:])
                    nc.vector.tensor_tensor(
                        out=ctxH[g * NUM_HEADS + h][:],
                        in0=ctx_ps[0:HEAD_DIM, :], in1=rdb[:],
                        op=mybir.AluOpType.mult)

                def outproj_dc(i, dc):
                    op = opsum.tile([128, 512], f32, tag="po", name="ps_o")
                    for h in range(NUM_HEADS):
                        nc.tensor.matmul(
                            op[:],
                            wot[:, h * D + dc * 128: h * D + dc * 128 + 128],
                            ctxH[i * NUM_HEADS + h][:],
                            start=(h == 0), stop=(h == NUM_HEADS - 1))
                    ost = opool.tile([128, 512], f32, tag="ost", name="ost")
                    nc.vector.tensor_scalar_add(ost[:], op[:],
                                                bo2t[:, dc:dc + 1])
                    nc.sync.dma_start(
                        out.ap()[dc * 128:(dc + 1) * 128,
                                 512 * i: 512 * (i + 1)], ost[:])

                # q chains one head ahead of group-0 attention
                for h in range(NUM_HEADS + 1):
                    if h < NUM_HEADS:
                        q_head(h)
                    if h > 0:
                        attention_head(0, h - 1)
                # group-1 attention interleaved with group-0 out-projection
                for h in range(NUM_HEADS):
                    attention_head(1, h)
                    if h % 2 == 1:
                        dc = (h - 1) // 2
                        outproj_dc(0, dc)
                for dc in range(4, KC):
                    outproj_dc(0, dc)
                for dc in range(KC):
                    outproj_dc(1, dc)
    nc.compile()
    return nc


_program_cache = {}


def _get_program():
    if "nc" not in _program_cache:
        _program_cache["nc"] = _build_program()
    return _program_cache["nc"]


def _host_masks():
    # Three mask patterns: d = key - token offset within the chunk window.
    # m0 (first chunk): d = kk - mm; m1/m2 (later chunks): d = kk - mm + 64.
    import ml_dtypes
    masks = []
    for (nk, nw, off) in ((128, 128, 0), (128, 192, HALO), (64, 64, HALO)):
        kk, mm = np.meshgrid(np.arange(nk), np.arange(nw), indexing="ij")
        d = kk - mm + off
        valid = (d >= 0) & (d <= HALO) & (d % 4 == 0) & (d != OVERLAP)
        masks.append(valid.astype(ml_dtypes.bfloat16))
    return masks


def kernel(main, begin, end, in_proj_w, in_proj_b, out_proj_w, out_proj_b):
    import ml_dtypes
    from concourse.bass_utils import run_bass_kernel_spmd

    bf = ml_dtypes.bfloat16
    main = np.asarray(main, np.float32)
    begin = np.asarray(begin, np.float32)
    end = np.asarray(end, np.float32)
    in_proj_w = np.asarray(in_proj_w, np.float32)
    in_proj_b = np.asarray(in_proj_b, np.float32)
    out_proj_w = np.asarray(out_proj_w, np.float32)
    out_proj_b = np.asarray(out_proj_b, np.float32)

    D = EMBED_DIM
    scale = HEAD_DIM ** -0.5
    wq, wk, wv = in_proj_w[:D], in_proj_w[D:2 * D], in_proj_w[2 * D:]
    bq_, bv = in_proj_b[:D], in_proj_b[2 * D:3 * D]
    combined = np.concatenate([begin, main, end], axis=0)  # [N + 64, D]

    f8 = ml_dtypes.float8_e4m3
    wqT = np.ascontiguousarray(wq.T * scale).astype(bf)
    wkT = np.ascontiguousarray(wk.T).astype(bf)
    wv8T = np.ascontiguousarray(wv.T * 64.0).astype(f8)
    woT = np.ascontiguousarray(out_proj_w.T).astype(bf)

    cst = np.zeros((128, NCONST), np.float32)
    cst[0:HEAD_DIM, 0:NUM_HEADS] = (bq_ * scale).reshape(NUM_HEADS, HEAD_DIM).T
    bo2 = out_proj_w @ bv + out_proj_b                      # [768]
    cst[:, 8:14] = bo2.reshape(KC, 128).T
    masks = _host_masks()
    mk = cst[:, 14:NCONST].view(bf)                         # [128, 384]
    mk[:, 0:128] = masks[0]
    mk[:, 128:320] = masks[1]
    mk[0:64, 320:384] = masks[2]

    shared = {"wqT": wqT, "wkT": wkT, "wv8T": wv8T, "woT": woT, "cst": cst}
    in_maps = []
    for c in range(N_CORES):
        xTc = np.ascontiguousarray(combined[c * TOK: c * TOK + ROWS].T)
        in_maps.append({**shared, "xT": xTc.astype(bf),
                        "x8T": xTc.astype(f8)})

    nc = _get_program()
    res = run_bass_kernel_spmd(nc, in_maps, core_ids=list(range(N_CORES)),
                               **_program_cache.get("run_kwargs", {}))
    _program_cache["last_result"] = res

    outp = np.empty((N_LINES, 2 * D), np.float32)
    outp[:, :D] = main
    for c in range(N_CORES):
        outp[c * TOK:(c + 1) * TOK, D:] = res.results[c]["out"].T
    return outp
